# revision 29
# baseline (speedup 1.0000x reference)
"""Trainium2 Bass kernel for nn_CapsuleLayer_45148696216021.

Mathematical structure (verified against the reference):
  caps = einsum('bi,nio->bno', x, rel_W) + rel_b          [B, N, O]
  caps_t[b] = caps[b].T.reshape(N, O)  (torch view quirk)
  u_hat[b,i,n] = sum_o caps_t[b,n,o] * rw[b,i,o]
  Dynamic routing with b_logits starting at 0: softmax over the capsule
  axis of a tensor whose rows (capsule axis) are identical stays exactly
  uniform (1/N) at EVERY iteration, because the agreement update
  b += einsum('bik,bjk->bji', u_hat, v) is j-independent when v rows are
  identical.  Hence the output v[b,j,:] == squash(sum_i u_hat[b,i,:]/N)
  for all j (bitwise identical rows in the reference too).

  sum_i u_hat[b,i,n] = sum_o caps_t[b,n,o] * rwsum[b,o]
  with rwsum[b,o] = sum_i rw[b,i,o].  Substituting the caps_t view:
  su[b,n] = sum_{r,m} caps[b,r,8n+m] * rwsum[b, m*128+r]

  So the only heavy compute is caps = x @ rel_W (34 GFLOP over 512 MB of
  weights), followed by a cheap weighted reduction.  rwsum and the rel_b
  bias contribution are tiny and computed on the host.

Sharding: the O axis (1024) is split into 8 slices of 128 columns; core d
computes caps[:, :, 128d:128d+128] for all relations, then reduces with
the rwsum weights to su[:, 16d:16d+16] fully on-chip (capsule n uses
exactly caps columns 8n..8n+7, which lie entirely in one slice).  The
only device output is su (8 KB/core); host applies bias + squash +
row-broadcast to the [128,128,128] output.
"""

import os
import sys
import tempfile
from concurrent.futures import ThreadPoolExecutor

import numpy as np

if "/opt/trn_rl_repo" not in sys.path:
    sys.path.insert(0, "/opt/trn_rl_repo")

import concourse.bass as bass
import concourse.mybir as mybir
import concourse.tile as tile
from concourse.vector_clock import ScopedClock
from concourse import bass_utils
from concourse.bass_utils import run_bass_kernel_spmd

if os.environ.get("BASS_LDW_OPT", "0") == "1":
    _orig_run_command = bass_utils.run_command

    def _patched_run_command(argv, **kw):
        argv = [
            "--enable-ldw-opt=true" if a == "--enable-ldw-opt=false" else a
            for a in argv
        ]
        return _orig_run_command(argv, **kw)

    bass_utils.run_command = _patched_run_command

B, I, O, N = 128, 1024, 1024, 128
NC = 8          # cores
G = 32          # relation groups of 4
CSL = O // NC   # 128 c-columns per core

_DT_NAME = os.environ.get("BASS_KERNEL_DTYPE", "bfloat16")
_DT = getattr(mybir.dt, _DT_NAME)
_DT_NP = {"float32": np.float32, "bfloat16": None, "float32r": np.float32}[_DT_NAME]
if _DT_NAME == "bfloat16":
    import ml_dtypes

    _DT_NP = ml_dtypes.bfloat16

LAST_RESULTS = None  # stashed BassKernelResults for test.py introspection


def _cheap_tail(self, tick_clock, wait_clock):
    """Minimal Tile kernel tail: gpsimd observes the global clock via a NOP
    wait chain (split to single waits later), then resets the semaphores for
    re-execution.  No drains / all-engine barriers: every proc's final tick
    is in the global clock, so nothing can touch a semaphore afterwards."""
    carrier = self.nc.gpsimd.nop(nofuse=True)
    wait_clock.add_sem_waits(
        carrier.ins, ScopedClock({None: tick_clock.global_clock})
    )
    popped = self.nc._tile_sem_poison_stack.pop()
    assert popped is self._sem_poison
    self.nc.clear_and_free_semaphores(list(self.sems.allocated().values()))


tile.TileContext._drain_and_barrier = _cheap_tail


def _strip_framework_overhead(nc):
    """Remove the bass preamble all-engine barrier + per-engine drains (a
    single-shot kernel reading no const-APs doesn't need them).  The
    reset-sema drain / range-clear of the tail is kept for re-execution."""
    n = 0
    for f in nc.m.functions:
        for blk in f.blocks:
            keep = []
            for inst in blk.instructions:
                tn = type(inst).__name__
                drop = False
                if tn == "InstDrain" and inst.reset_range_start is None:
                    drop = True
                elif tn == "InstEventSemaphore" and inst.name.startswith(
                    "barrier_"
                ):
                    drop = True
                if drop:
                    n += 1
                else:
                    keep.append(inst)
            blk.instructions = keep
    return n


def _split_multi_waits(nc):
    """This walrus build only supports one semaphore wait per instruction.
    Tile's wait-assigner can attach several; split the extras onto
    same-engine NOPs inserted immediately before the instruction (same
    semantics: the engine blocks on each wait in turn)."""
    n_split = 0
    for f in nc.m.functions:
        for blk in f.blocks:
            new = []
            dirty = False
            for inst in blk.instructions:
                si = inst.sync_info
                waits = list(si.on_wait) if si is not None else []
                if len(waits) > 1:
                    dirty = True
                    n_split += 1
                    for w in waits[:-1]:
                        nop = mybir.InstNoOp(
                            name=nc.get_next_instruction_name(), ins=[], outs=[]
                        )
                        nop.engine = inst.engine
                        nop.sync_info = mybir.SyncInfo(on_wait=[w], on_update=[])
                        new.append(nop)
                    inst.sync_info = mybir.SyncInfo(
                        on_wait=[waits[-1]], on_update=list(si.on_update)
                    )
                new.append(inst)
            if dirty:
                blk.instructions = new
    return n_split

_NC_CACHE = {}
_F_PRE = int(os.environ.get("BASS_F_PRE", "24"))
_F_MID = int(os.environ.get("BASS_F_MID", "6"))


# Weighted round-robin for weight-group DMA queues, proportional to
# measured queue rates (sync/scalar HWDGE ~111 GB/s, gpsimd SWDGE ~94 GB/s).
def _make_wq():
    w = {0: 10, 1: 11, 2: 9}
    acc = {0: 0.0, 1: 0.0, 2: 0.0}
    out = []
    for _ in range(30):
        for q in (0, 1, 2):
            acc[q] += w[q] / 30.0
        q = max(acc, key=lambda k: acc[k])
        acc[q] -= 1.0
        out.append(q)
    return out


_wq = _make_wq()


def _build_bass():
    """Per-core program: caps matmul over this core's c-slice + weighted
    reduction to su[:, 16 local capsules]."""
    key = _DT_NAME
    if key in _NC_CACHE:
        return _NC_CACHE[key]

    f32 = mybir.dt.float32
    nc = bass.Bass("TRN2", target_bir_lowering=False)
    xt_d = nc.declare_dram_parameter("xt", [128, 8, 128], _DT, isOutput=False)
    w_d = nc.declare_dram_parameter("w", [G, 128, 4, 8, CSL], _DT, isOutput=False)
    rw_d = nc.declare_dram_parameter("rwsv", [128, 8, 128], f32, isOutput=False)
    su_d = nc.declare_dram_parameter("su", [128, 16], f32, isOutput=True)

    with tile.TileContext(nc) as tc:
        with (
            tc.tile_pool(name="const", bufs=1) as cpool,
            tc.tile_pool(name="wts", bufs=8) as wpool,
            tc.tile_pool(name="tmp", bufs=3) as tpool,
            tc.tile_pool(name="ps", bufs=6, space="PSUM") as pspool,
            tc.tile_pool(name="warmp", bufs=1, space="PSUM") as warmpool,
        ):
            dma_engines = [nc.sync, nc.scalar, nc.gpsimd]
            # xt first (tiny) so warmup fillers can start immediately; then
            # group 0's weights split across all queues for the fastest
            # first-matmul; later groups ride one queue each (g%3 rotation).
            xt = cpool.tile([128, 8, 128], _DT)
            nc.sync.dma_start(xt[:], xt_d[:])
            rw = cpool.tile([128, 8, 128], f32)
            nc.scalar.dma_start(rw[:], rw_d[:])
            wt0 = wpool.tile([128, 4, 8, CSL], _DT, tag="wt")
            nc.gpsimd.dma_start(wt0[:, 0:2], w_d[0, :, 0:2])
            nc.sync.dma_start(wt0[:, 2:3], w_d[0, :, 2:3])
            nc.scalar.dma_start(wt0[:, 3:4], w_d[0, :, 3:4])
            wt1 = wpool.tile([128, 4, 8, CSL], _DT, tag="wt")
            nc.sync.dma_start(wt1[:, 0:2], w_d[1, :, 0:2])
            nc.scalar.dma_start(wt1[:, 2:4], w_d[1, :, 2:4])
            acc = cpool.tile([128, 4, 16, 8], f32)
            nc.vector.memset(acc[:], 0.0)

            # Scratch psum bank for HAM-warming filler matmuls (results unused)
            warm = warmpool.tile([128, 256], f32, tag="warm")

            def fillers(n):
                for _ in range(n):
                    nc.tensor.matmul(warm[:], xt[:, 0, :], xt[:, 0:2, :])

            # Warm the PE while the first weight tiles stream in
            fillers(_F_PRE)

            for g in range(G):
                if g == 0:
                    wt = wt0
                elif g == 1:
                    wt = wt1
                else:
                    wt = wpool.tile([128, 4, 8, CSL], _DT, tag="wt")
                    q = _wq[g - 2]
                    qa = dma_engines[q]
                    qb = dma_engines[(q + 1) % 3]
                    qa.dma_start(wt[:, 0:2], w_d[g, :, 0:2])
                    qb.dma_start(wt[:, 2:4], w_d[g, :, 2:4])
                ps = pspool.tile([128, 4, 16, 8], f32, tag="ps")
                for k in range(8):
                    nc.tensor.matmul(
                        ps[:],
                        xt[:, k, :],
                        wt[:, :, k, :],
                        start=(k == 0),
                        stop=(k == 7),
                    )
                # tmp = ps * rwsv[b, m, 4g+r4] (broadcast over nl)
                in1 = rw[:, :, 4 * g : 4 * g + 4].transpose([0, 2, 1])
                in1 = in1[:, :, None, :].to_broadcast([128, 4, 16, 8])
                tmp = tpool.tile([128, 4, 16, 8], f32, tag="tmp")
                nc.vector.tensor_tensor(tmp[:], ps[:], in1, mybir.AluOpType.mult)
                nc.vector.tensor_tensor(acc[:], acc[:], tmp[:], mybir.AluOpType.add)
                if g < G - 1:
                    fillers(_F_MID)

            su_t = cpool.tile([128, 16], f32)
            nc.vector.tensor_reduce(
                su_t[:],
                acc[:].transpose([0, 2, 1, 3]),
                mybir.AxisListType.XY,
                mybir.AluOpType.add,
            )
            nc.sync.dma_start(su_d[:], su_t[:])

    if os.environ.get("BASS_STRIP_FRAMEWORK", "1") == "1":
        _strip_framework_overhead(nc)
    _split_multi_waits(nc)
    _NC_CACHE[key] = nc
    return nc


def _to_bf16(a):
    """Fast float32 -> bfloat16 with round-to-nearest-even (numpy bit ops;
    ml_dtypes astype is ~50x slower)."""
    import ml_dtypes

    u = a.view(np.uint32)
    r = ((u >> 16) & 1) + np.uint32(0x7FFF)
    return ((u + r) >> 16).astype(np.uint16).view(ml_dtypes.bfloat16)


def _cast_dt(a):
    if _DT_NAME == "bfloat16":
        return _to_bf16(np.ascontiguousarray(a, np.float32))
    return np.ascontiguousarray(a, np.float32)


def _prep_core_w(rel_w6, d):
    # rel_w6: [G, 4, 8, 128, NC, CSL] view of rel_W -> (g, i_loc, r4, k, c)
    return _cast_dt(rel_w6[:, :, :, :, d, :].transpose(0, 3, 1, 2, 4))


def kernel(x, edge_index, edge_type, rel_W, rel_b, route_weights):
    global LAST_RESULTS
    x = np.asarray(x, np.float32)
    rel_W = np.asarray(rel_W, np.float32)
    rel_b = np.asarray(rel_b, np.float32)
    rw = np.asarray(route_weights, np.float32).reshape(B, I, O)

    # host-side tiny reductions
    rwsum = rw.sum(axis=1, dtype=np.float32)                # [B, O]
    rwsv = np.ascontiguousarray(rwsum.reshape(B, 8, 128))   # [b, m, r]
    bias2 = np.einsum(
        "rnm,bmr->bn", rel_b.reshape(N, N, 8), rwsv, optimize=True
    )  # [B, N]

    # device input prep
    xt = _cast_dt(x.reshape(B, 8, 128).transpose(2, 1, 0))  # [i_loc, k, b]
    rel_w6 = rel_W.reshape(G, 4, 8, 128, NC, CSL)  # (g, r4, k, i_loc, d, c)
    with ThreadPoolExecutor(NC) as ex:
        w_cores = list(ex.map(lambda d: _prep_core_w(rel_w6, d), range(NC)))

    nc = _build_bass()
    in_maps = [{"xt": xt, "w": w_cores[d], "rwsv": rwsv} for d in range(NC)]
    trace = bool(int(os.environ.get("KERNEL_TRACE", "0")))
    kwargs = {}
    if trace:
        kwargs["tmpdir"] = os.environ.get("KERNEL_TRACE_DIR") or tempfile.mkdtemp(
            prefix="capsule_trace_"
        )
    res = run_bass_kernel_spmd(nc, in_maps, list(range(NC)), trace=trace, **kwargs)
    LAST_RESULTS = res

    su = np.concatenate(
        [res.results[d]["su"] for d in range(NC)], axis=1
    )  # [B, N]
    su += bias2

    s = su * np.float32(1.0 / N)
    sn = np.sum(s * s, axis=-1, keepdims=True)
    vrow = (sn / (1.0 + sn) * s / np.sqrt(sn)).astype(np.float32)  # [B, N]
    out = np.empty((B, N, N), np.float32)
    out[:] = vrow[:, None, :]
    return out


# revision 30
# speedup vs baseline: 1.1099x; 1.1099x over previous
"""Trainium2 Bass kernel for nn_CapsuleLayer_45148696216021.

Mathematical structure (verified against the reference):
  caps = einsum('bi,nio->bno', x, rel_W) + rel_b          [B, N, O]
  caps_t[b] = caps[b].T.reshape(N, O)  (torch view quirk)
  u_hat[b,i,n] = sum_o caps_t[b,n,o] * rw[b,i,o]
  Dynamic routing with b_logits starting at 0: softmax over the capsule
  axis of a tensor whose rows (capsule axis) are identical stays exactly
  uniform (1/N) at EVERY iteration, because the agreement update
  b += einsum('bik,bjk->bji', u_hat, v) is j-independent when v rows are
  identical.  Hence the output v[b,j,:] == squash(sum_i u_hat[b,i,:]/N)
  for all j (bitwise identical rows in the reference too).

  sum_i u_hat[b,i,n] = sum_o caps_t[b,n,o] * rwsum[b,o]
  with rwsum[b,o] = sum_i rw[b,i,o].  Substituting the caps_t view:
  su[b,n] = sum_{r,m} caps[b,r,8n+m] * rwsum[b, m*128+r]

  So the only heavy compute is caps = x @ rel_W (34 GFLOP over 512 MB of
  weights), followed by a cheap weighted reduction.  rwsum and the rel_b
  bias contribution are tiny and computed on the host.

Sharding: the O axis (1024) is split into 8 slices of 128 columns; core d
computes caps[:, :, 128d:128d+128] for all relations, then reduces with
the rwsum weights to su[:, 16d:16d+16] fully on-chip (capsule n uses
exactly caps columns 8n..8n+7, which lie entirely in one slice).  The
only device output is su (8 KB/core); host applies bias + squash +
row-broadcast to the [128,128,128] output.
"""

import os
import sys
import tempfile
from concurrent.futures import ThreadPoolExecutor

import numpy as np

if "/opt/trn_rl_repo" not in sys.path:
    sys.path.insert(0, "/opt/trn_rl_repo")

import concourse.bass as bass
import concourse.mybir as mybir
import concourse.tile as tile
from concourse.vector_clock import ScopedClock
from concourse import bass_utils
from concourse.bass_utils import run_bass_kernel_spmd

if os.environ.get("BASS_LDW_OPT", "0") == "1":
    _orig_run_command = bass_utils.run_command

    def _patched_run_command(argv, **kw):
        argv = [
            "--enable-ldw-opt=true" if a == "--enable-ldw-opt=false" else a
            for a in argv
        ]
        return _orig_run_command(argv, **kw)

    bass_utils.run_command = _patched_run_command

B, I, O, N = 128, 1024, 1024, 128
NC = 8          # cores
G = 32          # relation groups of 4
CSL = O // NC   # 128 c-columns per core

_DT_NAME = os.environ.get("BASS_KERNEL_DTYPE", "bfloat16")
_DT = getattr(mybir.dt, _DT_NAME)
_DT_NP = {"float32": np.float32, "bfloat16": None, "float32r": np.float32}[_DT_NAME]
if _DT_NAME == "bfloat16":
    import ml_dtypes

    _DT_NP = ml_dtypes.bfloat16

LAST_RESULTS = None  # stashed BassKernelResults for test.py introspection


def _cheap_tail(self, tick_clock, wait_clock):
    """Minimal Tile kernel tail: gpsimd observes the global clock via a NOP
    wait chain (split to single waits later), then resets the semaphores for
    re-execution.  No drains / all-engine barriers: every proc's final tick
    is in the global clock, so nothing can touch a semaphore afterwards."""
    carrier = self.nc.gpsimd.nop(nofuse=True)
    wait_clock.add_sem_waits(
        carrier.ins, ScopedClock({None: tick_clock.global_clock})
    )
    popped = self.nc._tile_sem_poison_stack.pop()
    assert popped is self._sem_poison
    self.nc.clear_and_free_semaphores(list(self.sems.allocated().values()))


tile.TileContext._drain_and_barrier = _cheap_tail


def _strip_framework_overhead(nc):
    """Remove the bass preamble all-engine barrier + per-engine drains (a
    single-shot kernel reading no const-APs doesn't need them).  The
    reset-sema drain / range-clear of the tail is kept for re-execution."""
    n = 0
    for f in nc.m.functions:
        for blk in f.blocks:
            keep = []
            for inst in blk.instructions:
                tn = type(inst).__name__
                drop = False
                if tn == "InstDrain" and inst.reset_range_start is None:
                    drop = True
                elif tn == "InstEventSemaphore" and inst.name.startswith(
                    "barrier_"
                ):
                    drop = True
                if drop:
                    n += 1
                else:
                    keep.append(inst)
            blk.instructions = keep
    return n


def _split_multi_waits(nc):
    """This walrus build only supports one semaphore wait per instruction.
    Tile's wait-assigner can attach several; split the extras onto
    same-engine NOPs inserted immediately before the instruction (same
    semantics: the engine blocks on each wait in turn)."""
    n_split = 0
    for f in nc.m.functions:
        for blk in f.blocks:
            new = []
            dirty = False
            for inst in blk.instructions:
                si = inst.sync_info
                waits = list(si.on_wait) if si is not None else []
                if len(waits) > 1:
                    dirty = True
                    n_split += 1
                    for w in waits[:-1]:
                        nop = mybir.InstNoOp(
                            name=nc.get_next_instruction_name(), ins=[], outs=[]
                        )
                        nop.engine = inst.engine
                        nop.sync_info = mybir.SyncInfo(on_wait=[w], on_update=[])
                        new.append(nop)
                    inst.sync_info = mybir.SyncInfo(
                        on_wait=[waits[-1]], on_update=list(si.on_update)
                    )
                new.append(inst)
            if dirty:
                blk.instructions = new
    return n_split

_NC_CACHE = {}
_F_PRE = int(os.environ.get("BASS_F_PRE", "24"))
_F_MID = int(os.environ.get("BASS_F_MID", "6"))


# Weighted round-robin for weight-group DMA queues, proportional to
# measured queue rates (sync/scalar HWDGE ~111 GB/s, gpsimd SWDGE ~94 GB/s).
def _make_wq():
    w = {0: 10, 1: 11, 2: 9}
    acc = {0: 0.0, 1: 0.0, 2: 0.0}
    out = []
    for _ in range(30):
        for q in (0, 1, 2):
            acc[q] += w[q] / 30.0
        q = max(acc, key=lambda k: acc[k])
        acc[q] -= 1.0
        out.append(q)
    return out


_wq = _make_wq()


def _build_bass():
    """Per-core program: caps matmul over this core's c-slice + weighted
    reduction to su[:, 16 local capsules]."""
    key = _DT_NAME
    if key in _NC_CACHE:
        return _NC_CACHE[key]

    f32 = mybir.dt.float32
    nc = bass.Bass("TRN2", target_bir_lowering=False)
    xt_d = nc.declare_dram_parameter("xt", [128, 8, 128], _DT, isOutput=False)
    w_d = nc.declare_dram_parameter("w", [G, 128, 4, 8, CSL], _DT, isOutput=False)
    rw_d = nc.declare_dram_parameter("rwsv", [128, 8, 128], f32, isOutput=False)
    su_d = nc.declare_dram_parameter("su", [128, 16], f32, isOutput=True)

    with tile.TileContext(nc) as tc:
        with (
            tc.tile_pool(name="const", bufs=1) as cpool,
            tc.tile_pool(name="wts", bufs=8) as wpool,
            tc.tile_pool(name="tmp", bufs=3) as tpool,
            tc.tile_pool(name="ps", bufs=6, space="PSUM") as pspool,
            tc.tile_pool(name="warmp", bufs=1, space="PSUM") as warmpool,
        ):
            dma_engines = [nc.sync, nc.scalar, nc.gpsimd]
            # xt first (tiny) so warmup fillers can start immediately.
            # Group 0 is TWO independent 2-relation tiles so its first
            # matmuls can start after a single 0.5 MB DMA lands.
            xt = cpool.tile([128, 8, 128], _DT)
            nc.sync.dma_start(xt[:], xt_d[:])
            rw = cpool.tile([128, 8, 128], f32)
            nc.scalar.dma_start(rw[:], rw_d[:])
            wt0a = wpool.tile([128, 2, 8, CSL], _DT, tag="wt")
            nc.sync.dma_start(wt0a[:], w_d[0, :, 0:2])
            wt0b = wpool.tile([128, 2, 8, CSL], _DT, tag="wt")
            nc.scalar.dma_start(wt0b[:], w_d[0, :, 2:4])
            wt1 = wpool.tile([128, 4, 8, CSL], _DT, tag="wt")
            nc.gpsimd.dma_start(wt1[:], w_d[1])
            acc = cpool.tile([128, 4, 16, 8], f32)
            nc.vector.memset(acc[:], 0.0)

            # Scratch psum bank for HAM-warming filler matmuls (results unused)
            warm = warmpool.tile([128, 256], f32, tag="warm")

            def fillers(n):
                for _ in range(n):
                    nc.tensor.matmul(warm[:], xt[:, 0, :], xt[:, 0:2, :])

            # Warm the PE while the first weight tiles stream in
            fillers(_F_PRE)

            for g in range(G):
                if g == 0:
                    # two 2-relation halves, each gated only on its own DMA
                    for h2 in range(2):
                        wth = wt0a if h2 == 0 else wt0b
                        ps2 = pspool.tile(
                            [128, 2, 16, 8], f32, tag="ps", name=f"ps0_{h2}"
                        )
                        for k in range(8):
                            nc.tensor.matmul(
                                ps2[:],
                                xt[:, k, :],
                                wth[:, :, k, :],
                                start=(k == 0),
                                stop=(k == 7),
                            )
                        in1 = rw[:, :, 2 * h2 : 2 * h2 + 2].transpose([0, 2, 1])
                        in1 = in1[:, :, None, :].to_broadcast([128, 2, 16, 8])
                        tmp2 = tpool.tile(
                            [128, 2, 16, 8], f32, tag="tmp", name=f"tmp0_{h2}"
                        )
                        nc.vector.tensor_tensor(
                            tmp2[:], ps2[:], in1, mybir.AluOpType.mult
                        )
                        nc.vector.tensor_tensor(
                            acc[:, 2 * h2 : 2 * h2 + 2],
                            acc[:, 2 * h2 : 2 * h2 + 2],
                            tmp2[:],
                            mybir.AluOpType.add,
                        )
                    fillers(_F_MID)
                    continue
                elif g == 1:
                    wt = wt1
                else:
                    wt = wpool.tile([128, 4, 8, CSL], _DT, tag="wt")
                    q = _wq[g - 2]
                    qa = dma_engines[q]
                    qb = dma_engines[(q + 1) % 3]
                    qa.dma_start(wt[:, 0:2], w_d[g, :, 0:2])
                    qb.dma_start(wt[:, 2:4], w_d[g, :, 2:4])
                ps = pspool.tile([128, 4, 16, 8], f32, tag="ps")
                for k in range(8):
                    nc.tensor.matmul(
                        ps[:],
                        xt[:, k, :],
                        wt[:, :, k, :],
                        start=(k == 0),
                        stop=(k == 7),
                    )
                # tmp = ps * rwsv[b, m, 4g+r4] (broadcast over nl)
                in1 = rw[:, :, 4 * g : 4 * g + 4].transpose([0, 2, 1])
                in1 = in1[:, :, None, :].to_broadcast([128, 4, 16, 8])
                tmp = tpool.tile([128, 4, 16, 8], f32, tag="tmp")
                nc.vector.tensor_tensor(tmp[:], ps[:], in1, mybir.AluOpType.mult)
                nc.vector.tensor_tensor(acc[:], acc[:], tmp[:], mybir.AluOpType.add)
                if g < G - 1:
                    fillers(_F_MID)

            su_t = cpool.tile([128, 16], f32)
            nc.vector.tensor_reduce(
                su_t[:],
                acc[:].transpose([0, 2, 1, 3]),
                mybir.AxisListType.XY,
                mybir.AluOpType.add,
            )
            nc.sync.dma_start(su_d[:], su_t[:])

    if os.environ.get("BASS_STRIP_FRAMEWORK", "1") == "1":
        _strip_framework_overhead(nc)
    _split_multi_waits(nc)
    _NC_CACHE[key] = nc
    return nc


def _to_bf16(a):
    """Fast float32 -> bfloat16 with round-to-nearest-even (numpy bit ops;
    ml_dtypes astype is ~50x slower)."""
    import ml_dtypes

    u = a.view(np.uint32)
    r = ((u >> 16) & 1) + np.uint32(0x7FFF)
    return ((u + r) >> 16).astype(np.uint16).view(ml_dtypes.bfloat16)


def _cast_dt(a):
    if _DT_NAME == "bfloat16":
        return _to_bf16(np.ascontiguousarray(a, np.float32))
    return np.ascontiguousarray(a, np.float32)


def _prep_core_w(rel_w6, d):
    # rel_w6: [G, 4, 8, 128, NC, CSL] view of rel_W -> (g, i_loc, r4, k, c)
    return _cast_dt(rel_w6[:, :, :, :, d, :].transpose(0, 3, 1, 2, 4))


def kernel(x, edge_index, edge_type, rel_W, rel_b, route_weights):
    global LAST_RESULTS
    x = np.asarray(x, np.float32)
    rel_W = np.asarray(rel_W, np.float32)
    rel_b = np.asarray(rel_b, np.float32)
    rw = np.asarray(route_weights, np.float32).reshape(B, I, O)

    # host-side tiny reductions
    rwsum = rw.sum(axis=1, dtype=np.float32)                # [B, O]
    rwsv = np.ascontiguousarray(rwsum.reshape(B, 8, 128))   # [b, m, r]
    bias2 = np.einsum(
        "rnm,bmr->bn", rel_b.reshape(N, N, 8), rwsv, optimize=True
    )  # [B, N]

    # device input prep
    xt = _cast_dt(x.reshape(B, 8, 128).transpose(2, 1, 0))  # [i_loc, k, b]
    rel_w6 = rel_W.reshape(G, 4, 8, 128, NC, CSL)  # (g, r4, k, i_loc, d, c)
    with ThreadPoolExecutor(NC) as ex:
        w_cores = list(ex.map(lambda d: _prep_core_w(rel_w6, d), range(NC)))

    nc = _build_bass()
    in_maps = [{"xt": xt, "w": w_cores[d], "rwsv": rwsv} for d in range(NC)]
    trace = bool(int(os.environ.get("KERNEL_TRACE", "0")))
    kwargs = {}
    if trace:
        kwargs["tmpdir"] = os.environ.get("KERNEL_TRACE_DIR") or tempfile.mkdtemp(
            prefix="capsule_trace_"
        )
    res = run_bass_kernel_spmd(nc, in_maps, list(range(NC)), trace=trace, **kwargs)
    LAST_RESULTS = res

    su = np.concatenate(
        [res.results[d]["su"] for d in range(NC)], axis=1
    )  # [B, N]
    su += bias2

    s = su * np.float32(1.0 / N)
    sn = np.sum(s * s, axis=-1, keepdims=True)
    vrow = (sn / (1.0 + sn) * s / np.sqrt(sn)).astype(np.float32)  # [B, N]
    out = np.empty((B, N, N), np.float32)
    out[:] = vrow[:, None, :]
    return out
